# revision 56
# baseline (speedup 1.0000x reference)
"""Trainium2 Bass kernel for nn_L1RegressionActionHead.

Data-parallel over batch: 16 batch items -> 8 cores x 2 items.
Activations are dim-major on chip ((d on partitions, tokens on free axis));
x / output are transposed on host.  RoPE uses host-permuted q/k weights so
rotate_half is a 2-DMA swap of 64-partition blocks.

v2 pipeline design (vs v1):
- Work is split into 4 "units" of 512 tokens ((batch, half)) and issued as a
  software pipeline so TensorE never drains (PE p-state stays at 2.4 GHz).
- Softmax denominators for all 8 heads of a unit are accumulated into one
  [8,512] PSUM tile via one-hot-column matmuls, then ONE Ln + ONE Exp
  (scale=-1) yields all reciprocals; the LayerNorm variance row rides the
  same Ln/Exp pair with scale=-0.5.  This kills the per-(h,c) exp/ln
  activation-table thrash (73 table loads = 94us in v1).
- Reciprocal/rstd rows are partition-broadcast via DRAM-bounce DMAs (idle
  engine) instead of ones-matmuls + scalar copies.
- LayerNorm is folded into the FFN GEMM: z = relu(rstd*(y@w') - mu*rstd*s
  + b_eff) with w' = diag(ln_g) @ w_ffn, s = colsum(w'), b_eff = ln_b @
  w_ffn + b_ffn (host-computed).  y is pre-scaled by rstd (cheap DVE mul)
  and the -mu*rstd*s rank-1 term is one extra matmul accumulated into the
  FFN PSUM group; the yn materialization pass of v1 disappears.
- o_proj bias + residual fuse into one scalar_tensor_tensor per tile.
"""

import math
import sys

import numpy as np

sys.path.insert(0, "/opt/trn_rl_repo")

import ml_dtypes  # noqa: E402

import concourse.bass as bass  # noqa: E402
import concourse.tile as tile  # noqa: E402
from concourse import bacc, mybir  # noqa: E402
from concourse.bass_utils import run_bass_kernel_spmd  # noqa: E402

BF16 = ml_dtypes.bfloat16
FP8 = ml_dtypes.float8_e4m3
F32 = mybir.dt.float32
BF = mybir.dt.bfloat16
F8 = mybir.dt.float8e4
AF = mybir.ActivationFunctionType
OP = mybir.AluOpType
DR = mybir.MatmulPerfMode.DoubleRow
QS = 16.0  # power-of-2 scale folded into fp8 q weights, undone in rope tables

DIM = 1024
HEADS = 8
HD = 128
B = 16
T = 1024
KT = 64
KA = 2
KV = KT + KA  # 66
LN_EPS = 1e-5
NCORES = 8
BPC = B // NCORES  # 2 batch items per core
P = 128
TK = DIM // P  # 8 k/d tiles
CW = 512  # chunk width (tokens per pipeline unit)
UNITS = [(0, 0), (0, 1), (1, 0), (1, 1)]  # (batch, half)

# de-interleave permutation within each head's 128 dims
_PERM_HEAD = np.concatenate([np.arange(0, HD, 2), np.arange(1, HD, 2)])
_PERM_FULL = np.concatenate([h * HD + _PERM_HEAD for h in range(HEADS)])

# weight order inside the "wcat" input tensor (slot 7 = g-folded w_ffn)
_WIDX = {"w_o": 0, "w_ffn": 1, "w_va": 2, "w_vt": 3}
# bias slots inside "bias_cat": per-partition [128, slot, ko]
_BIDX = {"b_qa": 0, "b_qt": 1, "b_ka": 2, "b_kt": 3, "b_o": 4, "b_eff": 5}

_CACHED = None  # compiled Bass program, built once per process
LAST_RESULTS = None  # BassKernelResults of the most recent run


def _build_program():
    nc = bacc.Bacc("TRN2", target_bir_lowering=False, debug=False,
                   enable_asserts=False)

    xt_d = nc.dram_tensor("xt", (P, BPC, TK, T), BF, kind="ExternalInput").ap()
    xt8_d = nc.dram_tensor("xt8", (P, BPC, TK, T), F8, kind="ExternalInput").ap()
    wq8_d = nc.dram_tensor("wq8", (2, P, TK, DIM), F8, kind="ExternalInput").ap()
    wk8_d = nc.dram_tensor("wk8", (2, P, TK, DIM), F8, kind="ExternalInput").ap()
    wv8_d = nc.dram_tensor("wv8", (2, P, TK, DIM), F8, kind="ExternalInput").ap()
    hcat_d = nc.dram_tensor("hcat", (P, TK, 2 * KV), F8, kind="ExternalInput").ap()
    wcat_d = nc.dram_tensor("wcat", (4, P, TK, DIM), BF, kind="ExternalInput").ap()
    hcatb_d = nc.dram_tensor("hcatb", (P, TK, 2 * KV), BF, kind="ExternalInput").ap()
    bias_d = nc.dram_tensor("bias_cat", (P, 8, TK), F32, kind="ExternalInput").ap()
    bv_d = nc.dram_tensor("bv_comb", (P, DIM), BF, kind="ExternalInput").ap()
    cosq_d = nc.dram_tensor("cosq", (P, T), BF, kind="ExternalInput").ap()
    sinq_d = nc.dram_tensor("sinq", (P, T), BF, kind="ExternalInput").ap()
    cosk_d = nc.dram_tensor("cosk", (P, 2 * KV), BF, kind="ExternalInput").ap()
    sink_d = nc.dram_tensor("sink", (P, 2 * KV), BF, kind="ExternalInput").ap()
    out_d = nc.dram_tensor("outt", (P, BPC, TK, T), BF, kind="ExternalOutput").ap()

    with tile.TileContext(nc) as tc:
        _trace(nc, tc, xt_d, xt8_d, wq8_d, wk8_d, wv8_d, hcat_d, hcatb_d,
               wcat_d, bias_d, bv_d, cosq_d, sinq_d, cosk_d, sink_d, out_d)
    nc.compile()
    return nc


def _trace(nc, tc, xt_d, xt8_d, wq8_d, wk8_d, wv8_d, hcat_d, hcatb_d,
           wcat_d, bias_d, bv_d, cosq_d, sinq_d, cosk_d, sink_d, out_d):
    import contextlib
    ctx = contextlib.ExitStack()
    with ctx:
        consts = ctx.enter_context(tc.tile_pool(name="consts", bufs=1))
        x8pool = ctx.enter_context(tc.tile_pool(name="x8pool", bufs=1))
        xrpool = ctx.enter_context(tc.tile_pool(name="xrpool", bufs=2))
        wq8pool = ctx.enter_context(tc.tile_pool(name="wq8pool", bufs=2))
        qpool = ctx.enter_context(tc.tile_pool(name="qpool", bufs=4))
        wpool = ctx.enter_context(tc.tile_pool(name="wpool", bufs=2))
        expool = ctx.enter_context(tc.tile_pool(name="expool", bufs=2))
        rcbp = ctx.enter_context(tc.tile_pool(name="rcbp", bufs=1))
        atpool = ctx.enter_context(tc.tile_pool(name="atpool", bufs=1))
        ypool = ctx.enter_context(tc.tile_pool(name="ypool", bufs=2))
        scr = ctx.enter_context(tc.tile_pool(name="scr", bufs=2))
        dnpool = ctx.enter_context(tc.tile_pool(name="dnpool", bufs=1))
        stpool = ctx.enter_context(tc.tile_pool(name="stpool", bufs=1))
        st2pool = ctx.enter_context(tc.tile_pool(name="st2pool", bufs=2))
        zpool = ctx.enter_context(tc.tile_pool(name="zpool", bufs=2))
        dscr = ctx.enter_context(tc.tile_pool(name="dscr", bufs=2, space="DRAM"))
        psA = ctx.enter_context(tc.tile_pool(name="psA", bufs=4, space="PSUM"))
        psDen = ctx.enter_context(tc.tile_pool(name="psDen", bufs=2, space="PSUM"))
        psStat = ctx.enter_context(tc.tile_pool(name="psStat", bufs=1, space="PSUM"))

        # ---- constants (issue order = DMA order; k-GEMM needs come first)
        hcat_sb = consts.tile([P, TK, 2 * KV], F8, tag="hcat")
        nc.sync.dma_start(hcat_sb[:], hcat_d[:])
        hcatb_sb = consts.tile([P, TK, 2 * KV], BF, tag="hcatb")
        wk8 = {}
        for ki, knm in ((1, "kt"), (0, "ka")):
            w8 = xrpool.tile([P, TK, DIM], F8, tag="xres", name=f"wk8_{knm}")
            nc.sync.dma_start(w8[:], wk8_d[ki, :, :, :])
            wk8[ki] = w8
        bias_sb = consts.tile([P, 8, TK], F32, tag="bias")
        nc.sync.dma_start(bias_sb[:], bias_d[:])
        cosk_sb = consts.tile([P, 2 * KV], BF, tag="cosk")
        nc.sync.dma_start(cosk_sb[:], cosk_d[:])
        sink_sb = consts.tile([P, 2 * KV], BF, tag="sink")
        nc.sync.dma_start(sink_sb[:], sink_d[:])
        bv_sb = consts.tile([P, DIM], BF, tag="bv")
        cosq_sb = consts.tile([P, T], BF, tag="cosq")
        sinq_sb = consts.tile([P, T], BF, tag="sinq")
        ones_col = consts.tile([P, 1], BF, tag="onesc")
        nc.vector.memset(ones_col[:], 1.0)
        eps_col = consts.tile([1, 1], F32, tag="eps")
        nc.vector.memset(eps_col[:], LN_EPS)
        # one-hot [66, 8] columns: eye_sb[0:66, h, :] has col h = 1
        eye_sb = consts.tile([P, HEADS, HEADS], BF, tag="eye")
        nc.vector.memset(eye_sb[:], 0.0)
        for h in range(HEADS):
            nc.vector.memset(eye_sb[0:KV, h, h:h + 1], 1.0)

        def load_w(wname, split=1):
            wt = wpool.tile([P, TK, DIM], BF, tag="w", name=f"w_{wname}")
            step = TK // split
            for s in range(split):
                sl = slice(s * step, (s + 1) * step)
                nc.sync.dma_start(wt[:, sl, :], wcat_d[_WIDX[wname], :, sl, :])
            return wt

        def bias_ap(bname, n):
            return bias_sb[:, _BIDX[bname], n:n + 1]

        def bc_n(src2d, nrep):
            # broadcast a (128, W) AP over an inserted middle dim of size nrep
            return bass.AP(tensor=src2d.tensor, offset=src2d.offset,
                           ap=[list(src2d.ap[0]), [0, nrep],
                               list(src2d.ap[-1])])

        def rope_slab(dst, s, tmp_pool, tmp_tag, cos2d, sin2d, width):
            # in-place rotate-half on half-slab s of a (128, TK, width) tile
            HS = TK // 2
            sl = slice(s * HS, (s + 1) * HS)
            sw = tmp_pool.tile([P, HS, width], BF, tag=tmp_tag,
                               name=f"{tmp_tag}{s}", bufs=2)
            nc.sync.dma_start(sw[0:64, :, :], dst[64:128, sl, :])
            nc.sync.dma_start(sw[64:128, :, :], dst[0:64, sl, :])
            nc.vector.tensor_mul(dst[:, sl, :], dst[:, sl, :],
                                 bc_n(cos2d, HS))
            nc.vector.tensor_mul(sw[:], sw[:], bc_n(sin2d, HS))
            nc.vector.tensor_add(dst[:, sl, :], dst[:, sl, :], sw[:])

        def rope_tile(dst, tmp_pool, tmp_tag, cos2d, sin2d, width):
            for s in range(2):
                rope_slab(dst, s, tmp_pool, tmp_tag, cos2d, sin2d, width)

        # ================= K projections + rope ========================
        # krot columns: [0:64]=task b0, [64:128]=task b1, [128:130]=ad b0,
        # [130:132]=ad b1
        krot = consts.tile([P, TK, 2 * KV], BF, tag="krot")
        for n in range(TK):
            ps = psA.tile([P, CW], F32, tag="ps")
            for j in range(TK // 2):
                pr = slice(2 * j, 2 * j + 2)
                nc.tensor.matmul(ps[:, 0:128], wk8[1][:, pr, n * P:(n + 1) * P],
                                 hcat_sb[:, pr, 0:128],
                                 start=(j == 0), stop=(j == TK // 2 - 1),
                                 perf_mode=DR, skip_group_check=True)
            nc.scalar.activation(krot[:, n, 0:128], ps[:, 0:128],
                                 AF.Identity, bias=bias_ap("b_kt", n),
                                 scale=1.0 / QS)
            ps2 = psA.tile([P, CW], F32, tag="ps")
            for j in range(TK // 2):
                pr = slice(2 * j, 2 * j + 2)
                nc.tensor.matmul(ps2[:, 0:4], wk8[0][:, pr, n * P:(n + 1) * P],
                                 hcat_sb[:, pr, 128:132],
                                 start=(j == 0), stop=(j == TK // 2 - 1),
                                 perf_mode=DR, skip_group_check=True)
            nc.scalar.activation(krot[:, n, 128:132], ps2[:, 0:4],
                                 AF.Identity, bias=bias_ap("b_ka", n),
                                 scale=1.0 / QS)
        rope_tile(krot[:, :, :], scr, "ksw", cosk_sb[:], sink_sb[:], 2 * KV)

        # ---- fp8 x and q weights for DoubleRow q GEMMs (b0 first) -----
        nc.sync.dma_start(cosq_sb[:], cosq_d[:])
        nc.sync.dma_start(sinq_sb[:], sinq_d[:])
        xt8_sb = x8pool.tile([P, BPC, TK, T], F8, tag="xt8")
        nc.sync.dma_start(xt8_sb[:, 0, :, :], xt8_d[:, 0, :, :])
        wq8 = {}
        for qi in range(2):
            w8 = wq8pool.tile([P, TK, DIM], F8, tag="wq8", name=f"wq8_{qi}")
            nc.sync.dma_start(w8[:], wq8_d[qi, :, :, :])
            wq8[qi] = w8
        # v-GEMM inputs dispatched up front (no pool-slot waits): hcatb,
        # bv, wvt/wva stream while k/q matmuls run
        nc.sync.dma_start(hcatb_sb[:], hcatb_d[:])
        nc.sync.dma_start(bv_sb[:], bv_d[:])
        wt_vt = load_w("w_vt")
        wt_va = load_w("w_va")

        def issue_v_and_tail_loads():
            # issued after stage0(u0): v GEMMs fill the rope-latency gap
            nonlocal vcomb, wt_o, wt_f
            wt, wta = wt_vt, wt_va
            for b in range(BPC):
                for c in range(2):
                    ps = psA.tile([P, CW], F32, tag="ps")
                    for k in range(TK):
                        nc.tensor.matmul(ps[0:64, :],
                                         hcatb_sb[:, k, b * 64:(b + 1) * 64],
                                         wt[:, k, c * CW:(c + 1) * CW],
                                         start=(k == 0), stop=(k == TK - 1),
                                         skip_group_check=True)
                        nc.tensor.matmul(ps[64:66, :],
                                         hcatb_sb[:, k, 128 + 2 * b:130 + 2 * b],
                                         wta[:, k, c * CW:(c + 1) * CW],
                                         start=(k == 0), stop=(k == TK - 1),
                                         skip_group_check=True)
                    nc.vector.tensor_add(vcomb[0:KV, b, c * CW:(c + 1) * CW],
                                         ps[0:KV, :],
                                         bv_sb[0:KV, c * CW:(c + 1) * CW])
            nc.sync.dma_start(xt8_sb[:, 1, :, :], xt8_d[:, 1, :, :])
            wt_o = load_w("w_o")
            wt_f = load_w("w_ffn")  # g-folded w'

        # vcomb rows: [0:64]=task tokens, [64:66]=adapter tokens
        vcomb = consts.tile([P, BPC, DIM], BF, tag="vcomb")
        wt_o = wt_f = None

        # ================= software pipeline over 4 units ==============
        # state per unit held across steps
        q_rot = {}    # (qi, unit) -> [P, 8, 512] bf16
        xres = {}     # unit -> [P, 8, 512] bf16 residual slice
        EX = {}       # exp'd scores  [P(66 used), 8, 512] bf16
        dn_ps = {}    # denominator   [8, 512] f32 PSUM
        den_sb = {}   # denominator   [8, 512] bf16 SBUF
        rcb_dram = {}  # [8, 512] bf16 DRAM bounce
        rstd_dram = {}  # [1, 512] bf16 DRAM bounce
        var_sb = {}   # [1, 512] f32
        mu_sb = {}    # [1, 512] f32
        y_t = {}      # y = o_proj + x  [P, 8, 512] bf16
        ys_ps = {}
        yq_ps = {}
        rstd_bc = {}  # [P, 512] bf16 broadcast
        nmurstd = {}  # [1, 512] bf16

        def stage0(u):
            """fp8 DoubleRow q projections + rope for unit u."""
            b, ch = UNITS[u]
            cs = slice(ch * CW, (ch + 1) * CW)
            for qi, bname in ((0, "b_qa"), (1, "b_qt")):
                qt_t = qpool.tile([P, TK, CW], BF, tag="qbuf",
                                  name=f"q{qi}_u{u}")
                q_rot[(qi, u)] = qt_t
                for n in range(TK):
                    ps = psA.tile([P, CW], F32, tag="ps", name=f"q{qi}{u}_{n}")
                    for j in range(TK // 2):
                        nc.tensor.matmul(
                            ps[:], wq8[qi][:, 2 * j:2 * j + 2, n * P:(n + 1) * P],
                            xt8_sb[:, b, 2 * j:2 * j + 2, cs],
                            start=(j == 0), stop=(j == TK // 2 - 1),
                            perf_mode=DR, skip_group_check=True)
                    nc.scalar.activation(
                        qt_t[:, n, :], ps[:],
                        AF.Identity, bias=bias_ap(bname, n), scale=1.0 / QS)
                    if n == TK // 2 - 1:
                        rope_slab(qt_t, 0, scr, "qsw", cosq_sb[:, cs],
                                  sinq_sb[:, cs], CW)
                rope_slab(qt_t, 1, scr, "qsw", cosq_sb[:, cs],
                          sinq_sb[:, cs], CW)

        def stage1(u):
            """scores -> exp -> denominator accumulation for unit u."""
            b, ch = UNITS[u]
            cs = slice(ch * CW, (ch + 1) * CW)
            # stream the bf16 residual slice for stage3
            xr = xrpool.tile([P, TK, CW], BF, tag="xres", name=f"xres{u}")
            xres[u] = xr
            nc.sync.dma_start(xr[:], xt_d[:, b, :, cs])
            ex = expool.tile([P, HEADS, CW], BF, tag="ex", name=f"ex{u}")
            EX[u] = ex
            dps = psDen.tile([HEADS, CW], F32, tag="dn", name=f"dn{u}")
            dn_ps[u] = dps
            for h in range(HEADS):
                scps = psA.tile([P, CW], F32, tag="ps", name=f"sc{u}_{h}")
                nc.tensor.matmul(scps[0:64, :], krot[:, h, b * 64:(b + 1) * 64],
                                 q_rot[(1, u)][:, h, :], start=True, stop=True,
                                 skip_group_check=True)
                nc.tensor.matmul(scps[64:66, :],
                                 krot[:, h, 128 + 2 * b:130 + 2 * b],
                                 q_rot[(0, u)][:, h, :], start=True, stop=True,
                                 skip_group_check=True)
                nc.scalar.activation(ex[0:KV, h, :], scps[0:KV, :], AF.Exp)
                nc.tensor.matmul(dps[:], eye_sb[0:KV, h, :], ex[0:KV, h, :],
                                 start=(h == 0), stop=(h == HEADS - 1),
                                 skip_group_check=True)
            dsb = dnpool.tile([HEADS, CW], BF, tag="densb", name=f"den{u}")
            den_sb[u] = dsb
            nc.scalar.activation(dsb[:], dps[:], AF.Identity, scale=1.0)

        def recip_step(du, vu):
            """One Ln + one Exp serving unit du's softmax denominators
            (scale -1) and unit vu's LN variance (scale -0.5)."""
            if du is not None:
                lnd = dnpool.tile([HEADS, CW], F32, tag="lnd", name=f"lnd{du}")
                nc.scalar.activation(lnd[:], den_sb[du][:], AF.Ln)
            if vu is not None:
                lnv = stpool.tile([1, CW], F32, tag="lnv", name=f"lnv{vu}")
                nc.scalar.activation(lnv[:], var_sb[vu][:], AF.Ln,
                                     bias=eps_col[:], scale=1.0)
            if du is not None:
                rr = dnpool.tile([HEADS, CW], BF, tag="rr", name=f"rr{du}")
                nc.scalar.activation(rr[:], lnd[:], AF.Exp, scale=-1.0)
                drt = dscr.tile([HEADS, CW], BF, tag="rcbd", name=f"rcbd{du}")
                rcb_dram[du] = drt
                nc.sync.dma_start(drt[:], rr[:])
            if vu is not None:
                rs = stpool.tile([1, CW], BF, tag="rs", name=f"rs{vu}")
                nc.scalar.activation(rs[:], lnv[:], AF.Exp, scale=-0.5)
                drs = dscr.tile([1, CW], BF, tag="rstdd", name=f"rstdd{vu}")
                rstd_dram[vu] = drs
                nc.sync.dma_start(drs[:], rs[:])
                # broadcast rstd row to all 128 partitions via DRAM bounce
                rb = st2pool.tile([P, CW], BF, tag="rstdbc", name=f"rb{vu}")
                rstd_bc[vu] = rb
                nc.sync.dma_start(
                    rb[:], bass.AP(tensor=drs.tensor, offset=drs.offset,
                                   ap=[[0, P]] + list(drs.ap[1:])))
                # mu * rstd row, broadcast the same way (column-subtract in S5)
                mm_ = stpool.tile([1, CW], BF, tag="mmr", name=f"mmr{vu}")
                nc.vector.scalar_tensor_tensor(mm_[:], mu_sb[vu][:], 1.0,
                                               rs[:], OP.mult, OP.mult)
                drm = dscr.tile([1, CW], BF, tag="murd", name=f"murd{vu}")
                nc.sync.dma_start(drm[:], mm_[:])
                mb_ = st2pool.tile([P, CW], BF, tag="murbc", name=f"mb{vu}")
                nmurstd[vu] = mb_
                nc.sync.dma_start(
                    mb_[:], bass.AP(tensor=drm.tensor, offset=drm.offset,
                                    ap=[[0, P]] + list(drm.ap[1:])))

        def stage2(u):
            """normalize exp'd scores, attention output matmuls."""
            b, ch = UNITS[u]
            at = atpool.tile([P, HEADS, CW], BF, tag="attn", name=f"attn{u}")
            # one broadcast DMA: rcb_dram[u] (8, 512) -> (KV, 8, 512)
            rcb = rcbp.tile([KV, HEADS, CW], BF, tag="rcb", name=f"rcb{u}")
            src = rcb_dram[u][:]
            nc.sync.dma_start(
                rcb[:], bass.AP(tensor=src.tensor, offset=src.offset,
                                ap=[[0, KV]] + [list(d) for d in src.ap]))
            for h in range(HEADS):
                nc.gpsimd.tensor_mul(EX[u][0:KV, h, :], EX[u][0:KV, h, :],
                                     rcb[:, h, :])
                ops_ = psA.tile([P, CW], F32, tag="ps", name=f"ou{u}_{h}")
                nc.tensor.matmul(ops_[:], vcomb[0:KV, b, h * P:(h + 1) * P],
                                 EX[u][0:KV, h, :], start=True, stop=True,
                                 skip_group_check=True)
                nc.scalar.activation(at[:, h, :], ops_[:], AF.Identity,
                                     scale=1.0)
            return at

        def stage3(u, at):
            """o_proj + residual + LN statistics for unit u."""
            b, ch = UNITS[u]
            cs = slice(ch * CW, (ch + 1) * CW)
            yt = ypool.tile([P, TK, CW], BF, tag="y", name=f"y{u}")
            y_t[u] = yt
            ys = psStat.tile([1, CW], F32, tag="ys", name=f"ys{u}")
            yq = psStat.tile([1, CW], F32, tag="yq", name=f"yq{u}")
            ys_ps[u], yq_ps[u] = ys, yq
            for n in range(TK):
                ps = psA.tile([P, CW], F32, tag="ps", name=f"o{u}_{n}")
                for k in range(TK):
                    nc.tensor.matmul(ps[:], wt_o[:, k, n * P:(n + 1) * P],
                                     at[:, k, :],
                                     start=(k == 0), stop=(k == TK - 1),
                                     skip_group_check=True)
                # y = (o_psum + b_o) + x_residual in one DVE op
                nc.vector.scalar_tensor_tensor(
                    yt[:, n, :], ps[:], bias_ap("b_o", n),
                    xres[u][:, n, :], OP.add, OP.add)
                nc.tensor.matmul(ys[:], ones_col[:], yt[:, n, :],
                                 start=(n == 0), stop=(n == TK - 1),
                                 skip_group_check=True)
                ysq = scr.tile([P, CW], BF, tag="ysq", bufs=1)
                nc.vector.tensor_mul(ysq[:], yt[:, n, :], yt[:, n, :])
                nc.tensor.matmul(yq[:], ones_col[:], ysq[:],
                                 start=(n == 0), stop=(n == TK - 1),
                                 skip_group_check=True)
            # mu and var = E[y^2]/1 - mu^2 in 3 DVE ops / 2 tiles
            mu = stpool.tile([1, CW], F32, tag="mu", name=f"mu{u}")
            mu_sb[u] = mu
            nc.vector.tensor_scalar_mul(mu[:], ys[:], 1.0 / DIM)
            var = stpool.tile([1, CW], F32, tag="var", name=f"var{u}")
            var_sb[u] = var
            nc.vector.scalar_tensor_tensor(var[:], mu[:], -1.0, mu[:],
                                           OP.mult, OP.mult)  # -mu^2
            nc.vector.scalar_tensor_tensor(var[:], yq[:], 1.0 / DIM, var[:],
                                           OP.mult, OP.add)

        def stage5(u):
            """FFN with LN folded in: z = relu(yr@w' - mu*rstd*s + b_eff).
            yr = y*rstd computed in place into y_t[u]."""
            b, ch = UNITS[u]
            cs = slice(ch * CW, (ch + 1) * CW)
            yr = y_t[u]
            for n in range(TK):
                nc.vector.tensor_mul(yr[:, n, :], yr[:, n, :],
                                     rstd_bc[u][:])
                nc.vector.tensor_sub(yr[:, n, :], yr[:, n, :],
                                     nmurstd[u][:])
            for n in range(TK):
                ps = psA.tile([P, CW], F32, tag="ps", name=f"f{u}_{n}")
                for k in range(TK):
                    nc.tensor.matmul(ps[:], wt_f[:, k, n * P:(n + 1) * P],
                                     yr[:, k, :],
                                     start=(k == 0), stop=(k == TK - 1),
                                     skip_group_check=True)
                zt = zpool.tile([P, CW], BF, tag="z", name=f"z{u}_{n}")
                nc.scalar.activation(zt[:], ps[:], AF.Relu,
                                     bias=bias_ap("b_eff", n), scale=1.0)
                nc.sync.dma_start(out_d[:, b, n, cs], zt[:])

        # pipeline: step i issues S0(u_i) | S1(u_{i-1}) | S2,S3(u_{i-2}) |
        # recip(den u_{i-1}, var u_{i-2}) | S5(u_{i-3})
        NU = len(UNITS)
        for i in range(NU + 3):
            if 0 <= i - 3 < NU:
                stage5(i - 3)
            if i < NU:
                stage0(i)
            if i == 0:
                issue_v_and_tail_loads()
            if 0 <= i - 1 < NU:
                stage1(i - 1)
            if 0 <= i - 2 < NU:
                at = stage2(i - 2)
                stage3(i - 2, at)
            recip_step(i - 1 if 0 <= i - 1 < NU else None,
                       i - 2 if 0 <= i - 2 < NU else None)


# =====================  host-side preparation  =========================

def _rope_tables(L):
    inv = 1.0 / (10000.0 ** (np.arange(0, HD, 2, dtype=np.float32) / HD))
    freqs = np.arange(L, dtype=np.float32)[:, None] * inv[None, :]
    emb = np.concatenate([freqs, freqs], axis=-1)  # (L, 128)
    return np.cos(emb), np.sin(emb)


def _perm_tables(L, scale):
    cos, sin = _rope_tables(L)  # (L, 128)
    sign = np.concatenate([-np.ones(64, np.float32), np.ones(64, np.float32)])
    cosP = (cos[:, _PERM_HEAD].T * scale).astype(np.float32)      # (128, L)
    sinN = (sin[:, _PERM_HEAD].T * sign[:, None] * scale).astype(np.float32)
    return cosP, sinN


def _w_sb(w, permute):
    # (1024 k, 1024 n) -> (128 p, 8 ko, 1024 n) bf16, optional column perm
    if permute:
        w = w[:, _PERM_FULL]
    return np.ascontiguousarray(
        w.reshape(TK, P, DIM).transpose(1, 0, 2)).astype(BF16)


def _b_slot(bvec, permute):
    if permute:
        bvec = bvec[_PERM_FULL]
    return bvec.reshape(TK, P).T  # (128, 8)


def kernel(**inputs):
    global _CACHED
    if _CACHED is None:
        _CACHED = _build_program()
    nc = _CACHED

    inp = {k: np.asarray(v) for k, v in inputs.items()}
    x = inp["x"].astype(np.float32)
    h_a = inp["h_a"].astype(np.float32)
    h_t = inp["h_t"].astype(np.float32)
    p_in = inp["p"].astype(np.float32)
    ratio = 1.0 / (1.0 + np.exp(-np.float32(inp["g"][0])))  # sigmoid

    # fold LayerNorm affine into the FFN weights (host)
    w_ffn = inp["w_ffn"].astype(np.float32)
    ln_g = inp["ln_g"].astype(np.float32)
    ln_b = inp["ln_b"].astype(np.float32)
    w_ffn_g = ln_g[:, None] * w_ffn          # w' = diag(g) @ w_ffn
    b_eff = ln_b @ w_ffn + inp["b_ffn"].astype(np.float32)

    # weights (shared across cores); q/k/v weights ship as fp8 scaled by QS
    wcat = np.stack([_w_sb(inp["w_o"], False), _w_sb(w_ffn_g, False),
                     _w_sb(inp["w_va"], False), _w_sb(inp["w_vt"], False)])

    def _w8(name, permute):
        w = inp[name].astype(np.float32) * QS
        return _w_sb(w, permute).astype(FP8)

    wq8 = np.stack([_w8("w_qa", True), _w8("w_qt", True)])
    wk8 = np.stack([_w8("w_ka", True), _w8("w_kt", True)])
    wv8 = np.zeros((2, P, TK, DIM), FP8)  # unused (V runs bf16)
    zpad = np.zeros((DIM,), np.float32)
    bias_cat = np.stack([
        _b_slot(inp["b_qa"], True), _b_slot(inp["b_qt"], True),
        _b_slot(inp["b_ka"], True), _b_slot(inp["b_kt"], True),
        _b_slot(inp["b_o"], False), _b_slot(b_eff, False),
        _b_slot(zpad, False), _b_slot(zpad, False)],
        axis=1).astype(np.float32)  # (128, 8slots, 8ko)
    bv_comb = np.zeros((P, DIM), np.float32)
    bv_comb[0:KT, :] = inp["b_vt"][None, :]
    bv_comb[KT:KV, :] = inp["b_va"][None, :]
    bv_comb = bv_comb.astype(BF16)

    cosq, sinq = _perm_tables(T, np.float32(1.0 / math.sqrt(HD)))
    coskt, sinkt = _perm_tables(KT, ratio)
    coska, sinka = _perm_tables(KA, np.float32(1.0))
    cosk = np.concatenate([coskt, coskt, coska, coska], axis=1)  # (128, 132)
    sink = np.concatenate([sinkt, sinkt, sinka, sinka], axis=1)

    shared = {
        "wcat": wcat, "wq8": wq8, "wk8": wk8, "wv8": wv8,
        "bias_cat": bias_cat, "bv_comb": bv_comb,
        "cosq": cosq.astype(BF16), "sinq": sinq.astype(BF16),
        "cosk": cosk.astype(BF16), "sink": sink.astype(BF16),
    }

    in_maps = []
    for core in range(NCORES):
        b0 = core * BPC
        xc = x[b0:b0 + BPC]  # (2, 1024, 1024)
        xtf = np.ascontiguousarray(
            xc.reshape(BPC, T, TK, P).transpose(3, 0, 2, 1))
        xt = xtf.astype(BF16)
        xt8 = xtf.astype(FP8)
        hcat = np.zeros((P, TK, 2 * KV), np.float32)
        for b in range(BPC):
            htT = h_t[b0 + b].T.reshape(TK, P, KT).transpose(1, 0, 2)
            hcat[:, :, b * KT:(b + 1) * KT] = htT
            had = np.stack([h_a[b0 + b, 0], p_in[b0 + b, 0]], axis=1)  # (1024,2)
            hcat[:, :, 2 * KT + b * KA:2 * KT + (b + 1) * KA] = (
                had.reshape(TK, P, KA).transpose(1, 0, 2))
        in_maps.append({"xt": xt, "xt8": xt8, "hcat": hcat.astype(FP8),
                        "hcatb": hcat.astype(BF16), **shared})

    res = run_bass_kernel_spmd(nc, in_maps, core_ids=list(range(NCORES)))
    global LAST_RESULTS
    LAST_RESULTS = res

    out = np.empty((B, T, DIM), np.float32)
    for core in range(NCORES):
        ot = np.asarray(res.results[core]["outt"]).astype(np.float32)
        out[core * BPC:(core + 1) * BPC] = (
            ot.transpose(1, 3, 2, 0).reshape(BPC, T, DIM))
    return out


# revision 57
# speedup vs baseline: 1.1671x; 1.1671x over previous
"""Trainium2 Bass kernel for nn_L1RegressionActionHead.

Data-parallel over batch: 16 batch items -> 8 cores x 2 items.
Activations are dim-major on chip ((d on partitions, tokens on free axis));
x / output are transposed on host.  RoPE uses host-permuted q/k weights so
rotate_half is a 2-DMA swap of 64-partition blocks.

v2 pipeline design (vs v1):
- Work is split into 4 "units" of 512 tokens ((batch, half)) and issued as a
  software pipeline so TensorE never drains (PE p-state stays at 2.4 GHz).
- Softmax denominators for all 8 heads of a unit are accumulated into one
  [8,512] PSUM tile via one-hot-column matmuls, then ONE Ln + ONE Exp
  (scale=-1) yields all reciprocals; the LayerNorm variance row rides the
  same Ln/Exp pair with scale=-0.5.  This kills the per-(h,c) exp/ln
  activation-table thrash (73 table loads = 94us in v1).
- Reciprocal/rstd rows are partition-broadcast via DRAM-bounce DMAs (idle
  engine) instead of ones-matmuls + scalar copies.
- LayerNorm is folded into the FFN GEMM: z = relu(rstd*(y@w') - mu*rstd*s
  + b_eff) with w' = diag(ln_g) @ w_ffn, s = colsum(w'), b_eff = ln_b @
  w_ffn + b_ffn (host-computed).  y is pre-scaled by rstd (cheap DVE mul)
  and the -mu*rstd*s rank-1 term is one extra matmul accumulated into the
  FFN PSUM group; the yn materialization pass of v1 disappears.
- o_proj bias + residual fuse into one scalar_tensor_tensor per tile.
"""

import math
import sys

import numpy as np

sys.path.insert(0, "/opt/trn_rl_repo")

import ml_dtypes  # noqa: E402

import concourse.bass as bass  # noqa: E402
import concourse.tile as tile  # noqa: E402
from concourse import bacc, mybir  # noqa: E402
from concourse.bass_utils import run_bass_kernel_spmd  # noqa: E402

BF16 = ml_dtypes.bfloat16
FP8 = ml_dtypes.float8_e4m3
F32 = mybir.dt.float32
BF = mybir.dt.bfloat16
F8 = mybir.dt.float8e4
AF = mybir.ActivationFunctionType
OP = mybir.AluOpType
DR = mybir.MatmulPerfMode.DoubleRow
QS = 16.0  # power-of-2 scale folded into fp8 q weights, undone in rope tables

DIM = 1024
HEADS = 8
HD = 128
B = 16
T = 1024
KT = 64
KA = 2
KV = KT + KA  # 66
LN_EPS = 1e-5
NCORES = 8
BPC = B // NCORES  # 2 batch items per core
P = 128
TK = DIM // P  # 8 k/d tiles
CW = 512  # chunk width (tokens per pipeline unit)
UNITS = [(0, 0), (0, 1), (1, 0), (1, 1)]  # (batch, half)

# de-interleave permutation within each head's 128 dims
_PERM_HEAD = np.concatenate([np.arange(0, HD, 2), np.arange(1, HD, 2)])
_PERM_FULL = np.concatenate([h * HD + _PERM_HEAD for h in range(HEADS)])

# weight order inside the "wcat" input tensor (slot 7 = g-folded w_ffn)
_WIDX = {"w_o": 0, "w_ffn": 1, "w_va": 2, "w_vt": 3}
# bias slots inside "bias_cat": per-partition [128, slot, ko]
_BIDX = {"b_qa": 0, "b_qt": 1, "b_ka": 2, "b_kt": 3, "b_o": 4, "b_eff": 5}

_CACHED = None  # compiled Bass program, built once per process
LAST_RESULTS = None  # BassKernelResults of the most recent run


def _build_program():
    nc = bacc.Bacc("TRN2", target_bir_lowering=False, debug=False,
                   enable_asserts=False)

    xt_d = nc.dram_tensor("xt", (P, BPC, TK, T), BF, kind="ExternalInput").ap()
    xt8_d = nc.dram_tensor("xt8", (P, BPC, TK, T), F8, kind="ExternalInput").ap()
    wq8_d = nc.dram_tensor("wq8", (2, P, TK, DIM), F8, kind="ExternalInput").ap()
    wk8_d = nc.dram_tensor("wk8", (2, P, TK, DIM), F8, kind="ExternalInput").ap()
    wv8_d = nc.dram_tensor("wv8", (2, P, TK, DIM), F8, kind="ExternalInput").ap()
    hcat_d = nc.dram_tensor("hcat", (P, TK, 2 * KV), F8, kind="ExternalInput").ap()
    wcat_d = nc.dram_tensor("wcat", (4, P, TK, DIM), BF, kind="ExternalInput").ap()
    hcatb_d = nc.dram_tensor("hcatb", (P, TK, 2 * KV), BF, kind="ExternalInput").ap()
    bias_d = nc.dram_tensor("bias_cat", (P, 8, TK), F32, kind="ExternalInput").ap()
    bv_d = nc.dram_tensor("bv_comb", (P, DIM), BF, kind="ExternalInput").ap()
    cosq_d = nc.dram_tensor("cosq", (P, T), BF, kind="ExternalInput").ap()
    sinq_d = nc.dram_tensor("sinq", (P, T), BF, kind="ExternalInput").ap()
    cosk_d = nc.dram_tensor("cosk", (P, 2 * KV), BF, kind="ExternalInput").ap()
    sink_d = nc.dram_tensor("sink", (P, 2 * KV), BF, kind="ExternalInput").ap()
    out_d = nc.dram_tensor("outt", (P, BPC, TK, T), BF, kind="ExternalOutput").ap()

    with tile.TileContext(nc) as tc:
        _trace(nc, tc, xt_d, xt8_d, wq8_d, wk8_d, wv8_d, hcat_d, hcatb_d,
               wcat_d, bias_d, bv_d, cosq_d, sinq_d, cosk_d, sink_d, out_d)
    nc.compile()
    return nc


def _trace(nc, tc, xt_d, xt8_d, wq8_d, wk8_d, wv8_d, hcat_d, hcatb_d,
           wcat_d, bias_d, bv_d, cosq_d, sinq_d, cosk_d, sink_d, out_d):
    import contextlib
    ctx = contextlib.ExitStack()
    with ctx:
        consts = ctx.enter_context(tc.tile_pool(name="consts", bufs=1))
        x8pool = ctx.enter_context(tc.tile_pool(name="x8pool", bufs=1))
        xrpool = ctx.enter_context(tc.tile_pool(name="xrpool", bufs=2))
        wq8pool = ctx.enter_context(tc.tile_pool(name="wq8pool", bufs=2))
        qpool = ctx.enter_context(tc.tile_pool(name="qpool", bufs=4))
        wpool = ctx.enter_context(tc.tile_pool(name="wpool", bufs=2))
        expool = ctx.enter_context(tc.tile_pool(name="expool", bufs=2))
        rcbp = ctx.enter_context(tc.tile_pool(name="rcbp", bufs=1))
        atpool = ctx.enter_context(tc.tile_pool(name="atpool", bufs=1))
        ypool = ctx.enter_context(tc.tile_pool(name="ypool", bufs=2))
        scr = ctx.enter_context(tc.tile_pool(name="scr", bufs=2))
        dnpool = ctx.enter_context(tc.tile_pool(name="dnpool", bufs=1))
        stpool = ctx.enter_context(tc.tile_pool(name="stpool", bufs=1))
        st2pool = ctx.enter_context(tc.tile_pool(name="st2pool", bufs=2))
        zpool = ctx.enter_context(tc.tile_pool(name="zpool", bufs=2))
        dscr = ctx.enter_context(tc.tile_pool(name="dscr", bufs=2, space="DRAM"))
        psA = ctx.enter_context(tc.tile_pool(name="psA", bufs=4, space="PSUM"))
        psDen = ctx.enter_context(tc.tile_pool(name="psDen", bufs=2, space="PSUM"))
        psStat = ctx.enter_context(tc.tile_pool(name="psStat", bufs=1, space="PSUM"))

        # ---- constants (issue order = DMA order; k-GEMM needs come first)
        hcat_sb = consts.tile([P, TK, 2 * KV], F8, tag="hcat")
        nc.sync.dma_start(hcat_sb[:], hcat_d[:])
        hcatb_sb = consts.tile([P, TK, 2 * KV], BF, tag="hcatb")
        wk8 = {}
        for ki, knm in ((1, "kt"), (0, "ka")):
            w8 = xrpool.tile([P, TK, DIM], F8, tag="xres", name=f"wk8_{knm}")
            nc.sync.dma_start(w8[:], wk8_d[ki, :, :, :])
            wk8[ki] = w8
        bias_sb = consts.tile([P, 8, TK], F32, tag="bias")
        nc.sync.dma_start(bias_sb[:], bias_d[:])
        cosk_sb = consts.tile([P, 2 * KV], BF, tag="cosk")
        nc.sync.dma_start(cosk_sb[:], cosk_d[:])
        sink_sb = consts.tile([P, 2 * KV], BF, tag="sink")
        nc.sync.dma_start(sink_sb[:], sink_d[:])
        bv_sb = consts.tile([P, DIM], BF, tag="bv")
        cosq_sb = consts.tile([P, T], BF, tag="cosq")
        sinq_sb = consts.tile([P, T], BF, tag="sinq")
        ones_col = consts.tile([P, 1], BF, tag="onesc")
        nc.vector.memset(ones_col[:], 1.0)
        eps_col = consts.tile([1, 1], F32, tag="eps")
        nc.vector.memset(eps_col[:], LN_EPS)
        # one-hot [66, 8] columns: eye_sb[0:66, h, :] has col h = 1
        eye_sb = consts.tile([P, HEADS, HEADS], BF, tag="eye")
        nc.vector.memset(eye_sb[:], 0.0)
        for h in range(HEADS):
            nc.vector.memset(eye_sb[0:KV, h, h:h + 1], 1.0)

        def load_w(wname, split=1):
            wt = wpool.tile([P, TK, DIM], BF, tag="w", name=f"w_{wname}")
            step = TK // split
            for s in range(split):
                sl = slice(s * step, (s + 1) * step)
                nc.sync.dma_start(wt[:, sl, :], wcat_d[_WIDX[wname], :, sl, :])
            return wt

        def bias_ap(bname, n):
            return bias_sb[:, _BIDX[bname], n:n + 1]

        def bc_n(src2d, nrep):
            # broadcast a (128, W) AP over an inserted middle dim of size nrep
            return bass.AP(tensor=src2d.tensor, offset=src2d.offset,
                           ap=[list(src2d.ap[0]), [0, nrep],
                               list(src2d.ap[-1])])

        def rope_slab(dst, s, tmp_pool, tmp_tag, cos2d, sin2d, width):
            # in-place rotate-half on half-slab s of a (128, TK, width) tile
            HS = TK // 2
            sl = slice(s * HS, (s + 1) * HS)
            sw = tmp_pool.tile([P, HS, width], BF, tag=tmp_tag,
                               name=f"{tmp_tag}{s}", bufs=2)
            nc.sync.dma_start(sw[0:64, :, :], dst[64:128, sl, :])
            nc.sync.dma_start(sw[64:128, :, :], dst[0:64, sl, :])
            nc.vector.tensor_mul(dst[:, sl, :], dst[:, sl, :],
                                 bc_n(cos2d, HS))
            nc.vector.tensor_mul(sw[:], sw[:], bc_n(sin2d, HS))
            nc.vector.tensor_add(dst[:, sl, :], dst[:, sl, :], sw[:])

        def rope_tile(dst, tmp_pool, tmp_tag, cos2d, sin2d, width):
            for s in range(2):
                rope_slab(dst, s, tmp_pool, tmp_tag, cos2d, sin2d, width)

        # ================= K projections + rope ========================
        # krot columns: [0:64]=task b0, [64:128]=task b1, [128:130]=ad b0,
        # [130:132]=ad b1
        krot = consts.tile([P, TK, 2 * KV], BF, tag="krot")
        for n in range(TK):
            ps = psA.tile([P, CW], F32, tag="ps")
            for j in range(TK // 2):
                pr = slice(2 * j, 2 * j + 2)
                nc.tensor.matmul(ps[:, 0:128], wk8[1][:, pr, n * P:(n + 1) * P],
                                 hcat_sb[:, pr, 0:128],
                                 start=(j == 0), stop=(j == TK // 2 - 1),
                                 perf_mode=DR, skip_group_check=True)
            nc.scalar.activation(krot[:, n, 0:128], ps[:, 0:128],
                                 AF.Identity, bias=bias_ap("b_kt", n),
                                 scale=1.0 / QS)
            ps2 = psA.tile([P, CW], F32, tag="ps")
            for j in range(TK // 2):
                pr = slice(2 * j, 2 * j + 2)
                nc.tensor.matmul(ps2[:, 0:4], wk8[0][:, pr, n * P:(n + 1) * P],
                                 hcat_sb[:, pr, 128:132],
                                 start=(j == 0), stop=(j == TK // 2 - 1),
                                 perf_mode=DR, skip_group_check=True)
            nc.scalar.activation(krot[:, n, 128:132], ps2[:, 0:4],
                                 AF.Identity, bias=bias_ap("b_ka", n),
                                 scale=1.0 / QS)
        rope_tile(krot[:, :, :], scr, "ksw", cosk_sb[:], sink_sb[:], 2 * KV)

        # ---- fp8 x and q weights for DoubleRow q GEMMs (b0 first) -----
        nc.sync.dma_start(cosq_sb[:], cosq_d[:])
        nc.sync.dma_start(sinq_sb[:], sinq_d[:])
        xt8_sb = x8pool.tile([P, BPC, TK, T], F8, tag="xt8")
        nc.sync.dma_start(xt8_sb[:, 0, :, :], xt8_d[:, 0, :, :])
        wq8 = {}
        for qi in range(2):
            w8 = wq8pool.tile([P, TK, DIM], F8, tag="wq8", name=f"wq8_{qi}")
            nc.sync.dma_start(w8[:], wq8_d[qi, :, :, :])
            wq8[qi] = w8
        # v-GEMM inputs dispatched up front (no pool-slot waits): hcatb,
        # bv, wvt/wva stream while k/q matmuls run
        nc.sync.dma_start(hcatb_sb[:], hcatb_d[:])
        nc.sync.dma_start(bv_sb[:], bv_d[:])
        wt_vt = load_w("w_vt")
        wt_va = load_w("w_va")

        def issue_v_and_tail_loads():
            # issued after stage0(u0): v GEMMs fill the rope-latency gap
            nonlocal vcomb, wt_o, wt_f
            wt, wta = wt_vt, wt_va
            for b in range(BPC):
                for c in range(2):
                    ps = psA.tile([P, CW], F32, tag="ps")
                    for k in range(TK):
                        nc.tensor.matmul(ps[0:64, :],
                                         hcatb_sb[:, k, b * 64:(b + 1) * 64],
                                         wt[:, k, c * CW:(c + 1) * CW],
                                         start=(k == 0), stop=(k == TK - 1),
                                         skip_group_check=True)
                        nc.tensor.matmul(ps[64:66, :],
                                         hcatb_sb[:, k, 128 + 2 * b:130 + 2 * b],
                                         wta[:, k, c * CW:(c + 1) * CW],
                                         start=(k == 0), stop=(k == TK - 1),
                                         skip_group_check=True)
                    nc.vector.tensor_add(vcomb[0:KV, b, c * CW:(c + 1) * CW],
                                         ps[0:KV, :],
                                         bv_sb[0:KV, c * CW:(c + 1) * CW])
            nc.sync.dma_start(xt8_sb[:, 1, :, :], xt8_d[:, 1, :, :])
            wt_o = load_w("w_o")
            wt_f = load_w("w_ffn")  # g-folded w'

        # vcomb rows: [0:64]=task tokens, [64:66]=adapter tokens
        vcomb = consts.tile([P, BPC, DIM], BF, tag="vcomb")
        wt_o = wt_f = None

        # ================= software pipeline over 4 units ==============
        # state per unit held across steps
        q_rot = {}    # (qi, unit) -> [P, 8, 512] bf16
        xres = {}     # unit -> [P, 8, 512] bf16 residual slice
        EX = {}       # exp'd scores  [P(66 used), 8, 512] bf16
        dn_ps = {}    # denominator   [8, 512] f32 PSUM
        den_sb = {}   # denominator   [8, 512] bf16 SBUF
        rcb_dram = {}  # [8, 512] bf16 DRAM bounce
        rstd_dram = {}  # [1, 512] bf16 DRAM bounce
        var_sb = {}   # [1, 512] f32
        mu_sb = {}    # [1, 512] f32
        y_t = {}      # y = o_proj + x  [P, 8, 512] bf16
        ys_ps = {}
        yq_ps = {}
        rstd_bc = {}  # [P, 512] bf16 broadcast
        nmurstd = {}  # [1, 512] bf16

        def stage0(u):
            """fp8 DoubleRow q projections + rope for unit u."""
            b, ch = UNITS[u]
            cs = slice(ch * CW, (ch + 1) * CW)
            for qi, bname in ((0, "b_qa"), (1, "b_qt")):
                qt_t = qpool.tile([P, TK, CW], BF, tag="qbuf",
                                  name=f"q{qi}_u{u}")
                q_rot[(qi, u)] = qt_t
                for n in range(TK):
                    ps = psA.tile([P, CW], F32, tag="ps", name=f"q{qi}{u}_{n}")
                    for j in range(TK // 2):
                        nc.tensor.matmul(
                            ps[:], wq8[qi][:, 2 * j:2 * j + 2, n * P:(n + 1) * P],
                            xt8_sb[:, b, 2 * j:2 * j + 2, cs],
                            start=(j == 0), stop=(j == TK // 2 - 1),
                            perf_mode=DR, skip_group_check=True)
                    nc.scalar.activation(
                        qt_t[:, n, :], ps[:],
                        AF.Identity, bias=bias_ap(bname, n), scale=1.0 / QS)
                    if n == TK // 2 - 1:
                        rope_slab(qt_t, 0, scr, "qsw", cosq_sb[:, cs],
                                  sinq_sb[:, cs], CW)
                rope_slab(qt_t, 1, scr, "qsw", cosq_sb[:, cs],
                          sinq_sb[:, cs], CW)

        def stage1(u):
            """scores -> exp -> denominator accumulation for unit u."""
            b, ch = UNITS[u]
            cs = slice(ch * CW, (ch + 1) * CW)
            # stream the bf16 residual slice for stage3
            xr = xrpool.tile([P, TK, CW], BF, tag="xres", name=f"xres{u}")
            xres[u] = xr
            nc.sync.dma_start(xr[:], xt_d[:, b, :, cs])
            ex = expool.tile([P, HEADS, CW], BF, tag="ex", name=f"ex{u}")
            EX[u] = ex
            dps = psDen.tile([HEADS, CW], F32, tag="dn", name=f"dn{u}")
            dn_ps[u] = dps
            for h in range(HEADS):
                scps = psA.tile([P, CW], F32, tag="ps", name=f"sc{u}_{h}")
                nc.tensor.matmul(scps[0:64, :], krot[:, h, b * 64:(b + 1) * 64],
                                 q_rot[(1, u)][:, h, :], start=True, stop=True,
                                 skip_group_check=True)
                nc.tensor.matmul(scps[64:66, :],
                                 krot[:, h, 128 + 2 * b:130 + 2 * b],
                                 q_rot[(0, u)][:, h, :], start=True, stop=True,
                                 skip_group_check=True)
                nc.scalar.activation(ex[0:KV, h, :], scps[0:KV, :], AF.Exp)
                nc.tensor.matmul(dps[:], eye_sb[0:KV, h, :], ex[0:KV, h, :],
                                 start=(h == 0), stop=(h == HEADS - 1),
                                 skip_group_check=True)
            dsb = dnpool.tile([HEADS, CW], BF, tag="densb", name=f"den{u}")
            den_sb[u] = dsb
            nc.scalar.activation(dsb[:], dps[:], AF.Identity, scale=1.0)

        def recip_step(du, vu):
            """One Ln + one Exp serving unit du's softmax denominators
            (scale -1) and unit vu's LN variance (scale -0.5)."""
            if du is not None:
                lnd = dnpool.tile([HEADS, CW], F32, tag="lnd", name=f"lnd{du}")
                nc.scalar.activation(lnd[:], den_sb[du][:], AF.Ln)
            if vu is not None:
                lnv = stpool.tile([1, CW], F32, tag="lnv", name=f"lnv{vu}")
                nc.scalar.activation(lnv[:], var_sb[vu][:], AF.Ln,
                                     bias=eps_col[:], scale=1.0)
            if du is not None:
                rr = dnpool.tile([HEADS, CW], BF, tag="rr", name=f"rr{du}")
                nc.scalar.activation(rr[:], lnd[:], AF.Exp, scale=-1.0)
                drt = dscr.tile([HEADS, CW], BF, tag="rcbd", name=f"rcbd{du}")
                rcb_dram[du] = drt
                nc.sync.dma_start(drt[:], rr[:])
            if vu is not None:
                rs = stpool.tile([1, CW], BF, tag="rs", name=f"rs{vu}")
                nc.scalar.activation(rs[:], lnv[:], AF.Exp, scale=-0.5)
                drs = dscr.tile([1, CW], BF, tag="rstdd", name=f"rstdd{vu}")
                rstd_dram[vu] = drs
                nc.sync.dma_start(drs[:], rs[:])
                # broadcast rstd row to all 128 partitions via DRAM bounce
                rb = st2pool.tile([P, CW], BF, tag="rstdbc", name=f"rb{vu}")
                rstd_bc[vu] = rb
                nc.sync.dma_start(
                    rb[:], bass.AP(tensor=drs.tensor, offset=drs.offset,
                                   ap=[[0, P]] + list(drs.ap[1:])))
                # mu * rstd row, broadcast the same way (column-subtract in S5)
                mm_ = stpool.tile([1, CW], BF, tag="mmr", name=f"mmr{vu}")
                nc.vector.scalar_tensor_tensor(mm_[:], mu_sb[vu][:], 1.0,
                                               rs[:], OP.mult, OP.mult)
                drm = dscr.tile([1, CW], BF, tag="murd", name=f"murd{vu}")
                nc.sync.dma_start(drm[:], mm_[:])
                mb_ = st2pool.tile([P, CW], BF, tag="murbc", name=f"mb{vu}")
                nmurstd[vu] = mb_
                nc.sync.dma_start(
                    mb_[:], bass.AP(tensor=drm.tensor, offset=drm.offset,
                                    ap=[[0, P]] + list(drm.ap[1:])))

        def stage2(u):
            """normalize exp'd scores, attention output matmuls."""
            b, ch = UNITS[u]
            at = atpool.tile([P, HEADS, CW], BF, tag="attn", name=f"attn{u}")
            # one broadcast DMA: rcb_dram[u] (8, 512) -> (KV, 8, 512)
            rcb = rcbp.tile([KV, HEADS, CW], BF, tag="rcb", name=f"rcb{u}")
            src = rcb_dram[u][:]
            nc.sync.dma_start(
                rcb[:], bass.AP(tensor=src.tensor, offset=src.offset,
                                ap=[[0, KV]] + [list(d) for d in src.ap]))
            for h in range(HEADS):
                nc.vector.tensor_mul(EX[u][0:KV, h, :], EX[u][0:KV, h, :],
                                     rcb[:, h, :])
                ops_ = psA.tile([P, CW], F32, tag="ps", name=f"ou{u}_{h}")
                nc.tensor.matmul(ops_[:], vcomb[0:KV, b, h * P:(h + 1) * P],
                                 EX[u][0:KV, h, :], start=True, stop=True,
                                 skip_group_check=True)
                nc.scalar.activation(at[:, h, :], ops_[:], AF.Identity,
                                     scale=1.0)
            return at

        def stage3(u, at):
            """o_proj + residual + LN statistics for unit u."""
            b, ch = UNITS[u]
            cs = slice(ch * CW, (ch + 1) * CW)
            yt = ypool.tile([P, TK, CW], BF, tag="y", name=f"y{u}")
            y_t[u] = yt
            ys = psStat.tile([1, CW], F32, tag="ys", name=f"ys{u}")
            yq = psStat.tile([1, CW], F32, tag="yq", name=f"yq{u}")
            ys_ps[u], yq_ps[u] = ys, yq
            for n in range(TK):
                ps = psA.tile([P, CW], F32, tag="ps", name=f"o{u}_{n}")
                for k in range(TK):
                    nc.tensor.matmul(ps[:], wt_o[:, k, n * P:(n + 1) * P],
                                     at[:, k, :],
                                     start=(k == 0), stop=(k == TK - 1),
                                     skip_group_check=True)
                # y = (o_psum + b_o) + x_residual in one DVE op
                nc.vector.scalar_tensor_tensor(
                    yt[:, n, :], ps[:], bias_ap("b_o", n),
                    xres[u][:, n, :], OP.add, OP.add)
                nc.tensor.matmul(ys[:], ones_col[:], yt[:, n, :],
                                 start=(n == 0), stop=(n == TK - 1),
                                 skip_group_check=True)
                ysq = scr.tile([P, CW], BF, tag="ysq", bufs=1)
                nc.vector.tensor_mul(ysq[:], yt[:, n, :], yt[:, n, :])
                nc.tensor.matmul(yq[:], ones_col[:], ysq[:],
                                 start=(n == 0), stop=(n == TK - 1),
                                 skip_group_check=True)
            # mu and var = E[y^2]/1 - mu^2 in 3 DVE ops / 2 tiles
            mu = stpool.tile([1, CW], F32, tag="mu", name=f"mu{u}")
            mu_sb[u] = mu
            nc.vector.tensor_scalar_mul(mu[:], ys[:], 1.0 / DIM)
            var = stpool.tile([1, CW], F32, tag="var", name=f"var{u}")
            var_sb[u] = var
            nc.vector.scalar_tensor_tensor(var[:], mu[:], -1.0, mu[:],
                                           OP.mult, OP.mult)  # -mu^2
            nc.vector.scalar_tensor_tensor(var[:], yq[:], 1.0 / DIM, var[:],
                                           OP.mult, OP.add)

        def stage5(u):
            """FFN with LN folded in: z = relu(yr@w' - mu*rstd*s + b_eff).
            yr = y*rstd computed in place into y_t[u]."""
            b, ch = UNITS[u]
            cs = slice(ch * CW, (ch + 1) * CW)
            yr = y_t[u]
            for n in range(TK):
                nc.vector.tensor_mul(yr[:, n, :], yr[:, n, :],
                                     rstd_bc[u][:])
                nc.vector.tensor_sub(yr[:, n, :], yr[:, n, :],
                                     nmurstd[u][:])
            for n in range(TK):
                ps = psA.tile([P, CW], F32, tag="ps", name=f"f{u}_{n}")
                for k in range(TK):
                    nc.tensor.matmul(ps[:], wt_f[:, k, n * P:(n + 1) * P],
                                     yr[:, k, :],
                                     start=(k == 0), stop=(k == TK - 1),
                                     skip_group_check=True)
                zt = zpool.tile([P, CW], BF, tag="z", name=f"z{u}_{n}")
                nc.scalar.activation(zt[:], ps[:], AF.Relu,
                                     bias=bias_ap("b_eff", n), scale=1.0)
                nc.sync.dma_start(out_d[:, b, n, cs], zt[:])

        # pipeline: step i issues S0(u_i) | S1(u_{i-1}) | S2,S3(u_{i-2}) |
        # recip(den u_{i-1}, var u_{i-2}) | S5(u_{i-3})
        NU = len(UNITS)
        for i in range(NU + 3):
            if i < NU:
                stage0(i)
            if i == 0:
                issue_v_and_tail_loads()
            if 0 <= i - 1 < NU:
                stage1(i - 1)
            if 0 <= i - 2 < NU:
                at = stage2(i - 2)
                stage3(i - 2, at)
            recip_step(i - 1 if 0 <= i - 1 < NU else None,
                       i - 2 if 0 <= i - 2 < NU else None)
            if 0 <= i - 3 < NU:
                stage5(i - 3)


# =====================  host-side preparation  =========================

def _rope_tables(L):
    inv = 1.0 / (10000.0 ** (np.arange(0, HD, 2, dtype=np.float32) / HD))
    freqs = np.arange(L, dtype=np.float32)[:, None] * inv[None, :]
    emb = np.concatenate([freqs, freqs], axis=-1)  # (L, 128)
    return np.cos(emb), np.sin(emb)


def _perm_tables(L, scale):
    cos, sin = _rope_tables(L)  # (L, 128)
    sign = np.concatenate([-np.ones(64, np.float32), np.ones(64, np.float32)])
    cosP = (cos[:, _PERM_HEAD].T * scale).astype(np.float32)      # (128, L)
    sinN = (sin[:, _PERM_HEAD].T * sign[:, None] * scale).astype(np.float32)
    return cosP, sinN


def _w_sb(w, permute):
    # (1024 k, 1024 n) -> (128 p, 8 ko, 1024 n) bf16, optional column perm
    if permute:
        w = w[:, _PERM_FULL]
    return np.ascontiguousarray(
        w.reshape(TK, P, DIM).transpose(1, 0, 2)).astype(BF16)


def _b_slot(bvec, permute):
    if permute:
        bvec = bvec[_PERM_FULL]
    return bvec.reshape(TK, P).T  # (128, 8)


def kernel(**inputs):
    global _CACHED
    if _CACHED is None:
        _CACHED = _build_program()
    nc = _CACHED

    inp = {k: np.asarray(v) for k, v in inputs.items()}
    x = inp["x"].astype(np.float32)
    h_a = inp["h_a"].astype(np.float32)
    h_t = inp["h_t"].astype(np.float32)
    p_in = inp["p"].astype(np.float32)
    ratio = 1.0 / (1.0 + np.exp(-np.float32(inp["g"][0])))  # sigmoid

    # fold LayerNorm affine into the FFN weights (host)
    w_ffn = inp["w_ffn"].astype(np.float32)
    ln_g = inp["ln_g"].astype(np.float32)
    ln_b = inp["ln_b"].astype(np.float32)
    w_ffn_g = ln_g[:, None] * w_ffn          # w' = diag(g) @ w_ffn
    b_eff = ln_b @ w_ffn + inp["b_ffn"].astype(np.float32)

    # weights (shared across cores); q/k/v weights ship as fp8 scaled by QS
    wcat = np.stack([_w_sb(inp["w_o"], False), _w_sb(w_ffn_g, False),
                     _w_sb(inp["w_va"], False), _w_sb(inp["w_vt"], False)])

    def _w8(name, permute):
        w = inp[name].astype(np.float32) * QS
        return _w_sb(w, permute).astype(FP8)

    wq8 = np.stack([_w8("w_qa", True), _w8("w_qt", True)])
    wk8 = np.stack([_w8("w_ka", True), _w8("w_kt", True)])
    wv8 = np.zeros((2, P, TK, DIM), FP8)  # unused (V runs bf16)
    zpad = np.zeros((DIM,), np.float32)
    bias_cat = np.stack([
        _b_slot(inp["b_qa"], True), _b_slot(inp["b_qt"], True),
        _b_slot(inp["b_ka"], True), _b_slot(inp["b_kt"], True),
        _b_slot(inp["b_o"], False), _b_slot(b_eff, False),
        _b_slot(zpad, False), _b_slot(zpad, False)],
        axis=1).astype(np.float32)  # (128, 8slots, 8ko)
    bv_comb = np.zeros((P, DIM), np.float32)
    bv_comb[0:KT, :] = inp["b_vt"][None, :]
    bv_comb[KT:KV, :] = inp["b_va"][None, :]
    bv_comb = bv_comb.astype(BF16)

    cosq, sinq = _perm_tables(T, np.float32(1.0 / math.sqrt(HD)))
    coskt, sinkt = _perm_tables(KT, ratio)
    coska, sinka = _perm_tables(KA, np.float32(1.0))
    cosk = np.concatenate([coskt, coskt, coska, coska], axis=1)  # (128, 132)
    sink = np.concatenate([sinkt, sinkt, sinka, sinka], axis=1)

    shared = {
        "wcat": wcat, "wq8": wq8, "wk8": wk8, "wv8": wv8,
        "bias_cat": bias_cat, "bv_comb": bv_comb,
        "cosq": cosq.astype(BF16), "sinq": sinq.astype(BF16),
        "cosk": cosk.astype(BF16), "sink": sink.astype(BF16),
    }

    in_maps = []
    for core in range(NCORES):
        b0 = core * BPC
        xc = x[b0:b0 + BPC]  # (2, 1024, 1024)
        xtf = np.ascontiguousarray(
            xc.reshape(BPC, T, TK, P).transpose(3, 0, 2, 1))
        xt = xtf.astype(BF16)
        xt8 = xtf.astype(FP8)
        hcat = np.zeros((P, TK, 2 * KV), np.float32)
        for b in range(BPC):
            htT = h_t[b0 + b].T.reshape(TK, P, KT).transpose(1, 0, 2)
            hcat[:, :, b * KT:(b + 1) * KT] = htT
            had = np.stack([h_a[b0 + b, 0], p_in[b0 + b, 0]], axis=1)  # (1024,2)
            hcat[:, :, 2 * KT + b * KA:2 * KT + (b + 1) * KA] = (
                had.reshape(TK, P, KA).transpose(1, 0, 2))
        in_maps.append({"xt": xt, "xt8": xt8, "hcat": hcat.astype(FP8),
                        "hcatb": hcat.astype(BF16), **shared})

    res = run_bass_kernel_spmd(nc, in_maps, core_ids=list(range(NCORES)))
    global LAST_RESULTS
    LAST_RESULTS = res

    out = np.empty((B, T, DIM), np.float32)
    for core in range(NCORES):
        ot = np.asarray(res.results[core]["outt"]).astype(np.float32)
        out[core * BPC:(core + 1) * BPC] = (
            ot.transpose(1, 3, 2, 0).reshape(BPC, T, DIM))
    return out
